# revision 50
# baseline (speedup 1.0000x reference)
"""Trainium2 Bass kernel: ViT-style LSA attention (per-head learnable scale,
diagonal self-token mask), data-parallel over batch across 8 NeuronCores.

Reference computation (per batch b of 64, N=197 tokens, D=384, H=8, DH=64):
    qkv = x @ w_qkv ; split q,k,v ; per-head scale on q@k^T scores ;
    diagonal masked to -9.9e8 ; softmax ; attn @ v ; concat heads @ w_out + b.

Sharding: batch 64 -> 8 cores x 8 batches. Weights replicated. No
collectives; host concatenates the per-core outputs.

Device dataflow per core, all TensorE matmuls bf16 (correctness gate is 2e-2
relative error; measured end-to-end bf16 error is ~0.85e-2), fp32 PSUM
accumulation. The whole kernel streams per batch pair / batch so every engine
pipelines; attention heads are processed in quads sharing PSUM banks:

  xT  [384,2N]  PE transposes per batch pair (bf16 PSUM, one evac per j-tile)
  qT,kT [512,2N] = Wq^T/Wk^T @ xT   (Wq pre-scaled on host by the LSA scale)
  v   [N,512]   natural layout per batch; xT slices as stationary operand,
                a constant-1 column appended per head for the softmax sums
  S^T [j,i]     per (b, head-quad): kT stationary, qT moving -> shared PSUM
  P^T = exp(S^T) * mask01  (diag zeroed; max-subtract skipped, |S| < ~20)
  attn-out^T [65,i] = [v|1]-stationary @ P^T  (row 64 = softmax denominator)
  attnT = out^T * reciprocal(denominator)  (broadcast via gpsimd from
                partition 0 — the ucode ignores AP partition offsets on HW)
  out [N,384] = attnT-stationary @ Wo + b

build_nc(reps=R) emits the body R times (per-rep PSUM pool scopes) so HW time
can be measured by wall-clock amplification — this container has no NTFF hook.
"""

import sys

sys.path.insert(0, "/opt/trn_rl_repo")

from contextlib import ExitStack

import ml_dtypes
import numpy as np

import concourse.bass as bass
import concourse.tile as tile
from concourse import bacc, mybir
from concourse.bass_utils import run_bass_kernel_spmd

BF16 = mybir.dt.bfloat16
F32 = mybir.dt.float32
NPBF16 = ml_dtypes.bfloat16

NCORES = 8
B_CORE = 8            # batches per core
N = 197               # tokens per batch
D = 384               # model dim
H = 8                 # heads
DH = 64               # head dim
INNER = H * DH        # 512
T = B_CORE * N        # 1576 tokens per core

# token-chunk starts/sizes for the moving operand of the q/k projections
CHUNKS = [(0, 512), (512, 512), (1024, 512), (1536, T - 1536)]
# per-batch key tiles: (offset, rows)
JTILES = [(0, 128), (128, N - 128)]
# j-tiles over a batch PAIR (transpose/projection granularity)
JTILES2 = [(0, 128), (128, 69), (197, 128), (325, 69)]
# output token tiles
TTILES = [(t0, min(128, T - t0)) for t0 in range(0, T, 128)]

EXP = mybir.ActivationFunctionType.Exp


def build_nc():
    nc = bacc.Bacc("TRN2", target_bir_lowering=False, debug=False)

    x = nc.dram_tensor("x", [T, D], BF16, kind="ExternalInput").ap()
    wq = nc.dram_tensor("wq", [D, INNER], BF16, kind="ExternalInput").ap()
    wk = nc.dram_tensor("wk", [D, INNER], BF16, kind="ExternalInput").ap()
    wv = nc.dram_tensor("wv", [D, INNER], BF16, kind="ExternalInput").ap()
    wo = nc.dram_tensor("wo", [INNER, D], BF16, kind="ExternalInput").ap()
    mask = nc.dram_tensor("mask01", [2, 128, 4, N], BF16, kind="ExternalInput").ap()
    iddr = nc.dram_tensor("ident", [128, 128], BF16, kind="ExternalInput").ap()
    bias = nc.dram_tensor("bias", [128, D], F32, kind="ExternalInput").ap()
    out = nc.dram_tensor("out", [T, D], F32, kind="ExternalOutput").ap()

    with tile.TileContext(nc) as tc, ExitStack() as ctx:
        const = ctx.enter_context(tc.tile_pool(name="const", bufs=1))

        xTs = [
            const.tile([128, T], BF16, tag=f"xT{i}", name=f"xT{i}") for i in range(3)
        ]
        ident = const.tile([128, 128], BF16)
        qT = const.tile([128, 4, T], BF16)
        kT = const.tile([128, 4, T], BF16)
        wq_sb = const.tile([128, 3, INNER], BF16)
        wk_sb = const.tile([128, 3, INNER], BF16)
        wv_sb = const.tile([128, 3, INNER], BF16)
        wo_sb = const.tile([128, 4, D], BF16)
        mk_sb = const.tile([128, 2, 4, N], BF16)
        bi_sb = const.tile([128, D], F32)

        # x tiles ride the SP queue (they gate the transposes feeding every
        # matmul); weights/mask/bias ride the Activation HWDGE queue in parallel
        # identity ships from the host: the gpsimd-built one (memset +
        # affine_select) gates the first PE transpose by ~2us at startup
        nc.scalar.dma_start(out=ident[:], in_=iddr[:])

        vv_pool = ctx.enter_context(tc.tile_pool(name="vv", bufs=3))
        pt_pool = ctx.enter_context(tc.tile_pool(name="pt", bufs=6))
        rp_pool = ctx.enter_context(tc.tile_pool(name="rp", bufs=6))
        rb_pool = ctx.enter_context(tc.tile_pool(name="rb", bufs=6))
        ob_pool = ctx.enter_context(tc.tile_pool(name="ob", bufs=3))

        # ---- x^T via PE transposes, then q/k projections: qT = (x @ Wq)^T ----
        # (own psum scope: its 4 banks are returned before the attention pools open)
        with (
            tc.tile_pool(name="qk_psum", bufs=4, space="PSUM") as qk_psum,
            tc.tile_pool(name="xsb", bufs=3) as x_pool,
        ):
            evac_flip = 0
            for tt, (t0, tsz) in enumerate(TTILES):
                xs = x_pool.tile([128, D], BF16, tag="xs")
                nc.sync.dma_start(out=xs[:tsz, :], in_=x[t0 : t0 + tsz, :])
                if tt == 3:
                    # q/k weights gate the first projections; the rest arrives
                    # before its consumers. One queue, ordered by need.
                    nc.sync.dma_start(
                        out=wq_sb[:], in_=wq.rearrange("(t p) n -> p t n", p=128)
                    )
                    nc.sync.dma_start(
                        out=wk_sb[:], in_=wk.rearrange("(t p) n -> p t n", p=128)
                    )
                if tt == 7:
                    nc.sync.dma_start(
                        out=wv_sb[:], in_=wv.rearrange("(t p) n -> p t n", p=128)
                    )
                if tt == 9:
                    nc.sync.dma_start(
                        out=wo_sb[:], in_=wo.rearrange("(t p) n -> p t n", p=128)
                    )
                if tt == 10:
                    nc.sync.dma_start(
                        out=mk_sb[:], in_=mask.rearrange("t p h n -> p t h n")
                    )
                    nc.sync.dma_start(out=bi_sb[:], in_=bias)
                for kt in range(3):
                    pxp = qk_psum.tile([128, 128], BF16, tag="qk")
                    nc.tensor.transpose(
                        pxp[:, :tsz],
                        xs[:tsz, kt * 128 : (kt + 1) * 128],
                        ident[:tsz, :tsz],
                    )
                    if evac_flip % 2 == 0:
                        nc.scalar.copy(xTs[kt][:, t0 : t0 + tsz], pxp[:, :tsz])
                    else:
                        nc.vector.tensor_copy(xTs[kt][:, t0 : t0 + tsz], pxp[:, :tsz])
                    evac_flip += 1
            for w_sb, dstT in ((wq_sb, qT), (wk_sb, kT)):
                for ft in range(4):
                    for c0, csz in CHUNKS:
                        ps = qk_psum.tile([128, 512], F32, tag="qk")
                        for kt in range(3):
                            nc.tensor.matmul(
                                ps[:, :csz],
                                lhsT=w_sb[:, kt, ft * 128 : (ft + 1) * 128],
                                rhs=xTs[kt][:, c0 : c0 + csz],
                                start=(kt == 0),
                                stop=(kt == 2),
                            )
                        if evac_flip % 2 == 0:
                            nc.scalar.copy(dstT[:, ft, c0 : c0 + csz], ps[:, :csz])
                        else:
                            nc.vector.tensor_copy(
                                dstT[:, ft, c0 : c0 + csz], ps[:, :csz]
                            )
                        evac_flip += 1

        v_psum = ctx.enter_context(tc.tile_pool(name="v_psum", bufs=1, space="PSUM"))
        o_psum = ctx.enter_context(tc.tile_pool(name="o_psum", bufs=1, space="PSUM"))
        d_psum = ctx.enter_context(tc.tile_pool(name="d_psum", bufs=2, space="PSUM"))
        a_psum = ctx.enter_context(tc.tile_pool(name="a_psum", bufs=2, space="PSUM"))
        at_pool = ctx.enter_context(tc.tile_pool(name="at", bufs=3))

        # ---- per-batch: v (natural layout, with ones column), then attention ----
        for b in range(B_CORE):
            t_b = b * N
            aT = at_pool.tile([128, 4, N], BF16, tag="at")
            vv = vv_pool.tile([128, 2, H * 65], BF16, tag="vv")
            for jt, (j0, jsz) in enumerate(JTILES):
                tok0 = t_b + j0
                pv = v_psum.tile([128, 512], F32, tag="v")
                for kt in range(3):
                    nc.tensor.matmul(
                        pv[:jsz, :],
                        lhsT=xTs[kt][:, tok0 : tok0 + jsz],
                        rhs=wv_sb[:, kt, :],
                        start=(kt == 0),
                        stop=(kt == 2),
                    )
                vj = vv[:jsz, jt].rearrange("p (h c) -> p h c", c=65)
                nc.gpsimd.memset(vj[:, :, 64:65], 1.0)
                nc.scalar.copy(
                    vj[:, :, 0:64], pv[:jsz, :].rearrange("p (h c) -> p h c", c=64)
                )

            # head quads sharing one partition half (po): heads (po-parity) with
            # feature tiles 0..3, so exp/mask/recip/broadcast/normalize each run
            # once per quad. PSUM quad tiles use a 256-element head stride so no
            # matmul output crosses a 2KB bank boundary; start/stop flags open
            # and close an accumulation group per bank.
            for quad in ((0, 2, 4, 6), (1, 3, 5, 7)):
                po = (quad[0] % 2) * 64
                pt = pt_pool.tile([128, 2, 4, N], BF16, tag="pt")
                for jt, (j0, jsz) in enumerate(JTILES):
                    tok0 = t_b + j0
                    pd = d_psum.tile([128, 4, 256], F32, tag="d")
                    for hh, h in enumerate(quad):
                        nc.tensor.matmul(
                            pd[:jsz, hh, :N],
                            lhsT=kT[po : po + 64, h // 2, tok0 : tok0 + jsz],
                            rhs=qT[po : po + 64, h // 2, t_b : t_b + N],
                            start=(hh % 2 == 0),
                            stop=(hh % 2 == 1),
                        )
                    nc.scalar.activation(pt[:jsz, jt], pd[:jsz, :, :N], EXP)
                    nc.gpsimd.tensor_mul(pt[:jsz, jt], pt[:jsz, jt], mk_sb[:jsz, jt])
                for pi in range(2):
                    fa = quad[2 * pi] // 2
                    pa = a_psum.tile([65, 2, N], F32, tag="a")
                    for jt, (j0, jsz) in enumerate(JTILES):
                        for hh in (2 * pi, 2 * pi + 1):
                            h = quad[hh]
                            nc.tensor.matmul(
                                pa[:, hh - 2 * pi, :],
                                lhsT=vv[:jsz, jt, h * 65 : (h + 1) * 65],
                                rhs=pt[:jsz, jt, hh, :],
                                start=(jt == 0 and hh % 2 == 0),
                                stop=(jt == 1 and hh % 2 == 1),
                            )
                    # reciprocal lands on partition 0: the partition_broadcast
                    # ucode ignores the AP partition offset (HW-verified)
                    rp = rp_pool.tile([1, 2, N], F32, tag="rp")
                    nc.vector.reciprocal(rp[0:1], pa[64:65])
                    rb = rb_pool.tile([64, 2, N], F32, tag="rb")
                    nc.gpsimd.partition_broadcast(rb[:], rp[0:1])
                    nc.vector.tensor_mul(
                        aT[po : po + 64, fa : fa + 2, :], pa[0:64], rb[:]
                    )

            # ---- output projection + bias for this batch (overlaps the next
            # batch's attention; aT tile is released back to the pool) ----
            for j0, jsz in JTILES:
                pp = o_psum.tile([128, D], F32, tag="o")
                for kf in range(4):
                    nc.tensor.matmul(
                        pp[:jsz, :],
                        lhsT=aT[:, kf, j0 : j0 + jsz],
                        rhs=wo_sb[:, kf, :],
                        start=(kf == 0),
                        stop=(kf == 3),
                    )
                ob = ob_pool.tile([128, D], F32, tag="ob")
                nc.vector.tensor_add(ob[:jsz, :], pp[:jsz, :], bi_sb[:jsz, :])
                nc.sync.dma_start(
                    out=out[t_b + j0 : t_b + j0 + jsz, :], in_=ob[:jsz, :]
                )

    return nc


_CACHE: dict = {}


def get_compiled():
    if "nc" not in _CACHE:
        nc = build_nc()
        nc.compile()
        _CACHE["nc"] = nc
    return _CACHE["nc"]


def make_in_maps(x, w_qkv, scale, w_out, b_out):
    x = np.asarray(x, np.float32)
    w_qkv = np.asarray(w_qkv, np.float32)
    scale = np.asarray(scale, np.float32)
    w_out = np.asarray(w_out, np.float32)
    b_out = np.asarray(b_out, np.float32)

    # fold the per-head LSA scale into Wq (exact in real arithmetic; the
    # scores become (x @ (Wq*s)) @ k^T = s * (q @ k^T))
    scale_rep = np.repeat(scale, DH)  # [512]
    wq = (w_qkv[:, :INNER] * scale_rep[None, :]).astype(NPBF16)
    wk = w_qkv[:, INNER : 2 * INNER].astype(NPBF16)
    wv = w_qkv[:, 2 * INNER :].astype(NPBF16)
    wo = w_out.astype(NPBF16)
    bias = np.ascontiguousarray(np.broadcast_to(b_out, (128, D)))

    mask = np.ones((2, 128, N), np.float32)
    for t in range(2):
        for j in range(128):
            g = t * 128 + j
            if g < N:
                mask[t, j, g] = 0.0
    # duplicated along a head-quad axis: one gpsimd multiply masks four heads
    mask = np.repeat(mask[:, :, None, :], 4, axis=2).astype(NPBF16)

    xs = x.reshape(NCORES, B_CORE, N, D)
    in_maps = []
    for c in range(NCORES):
        in_maps.append(
            {
                "x": np.ascontiguousarray(xs[c].reshape(T, D)).astype(NPBF16),
                "wq": wq,
                "wk": wk,
                "wv": wv,
                "wo": wo,
                "mask01": mask,
                "bias": bias,
                "ident": np.eye(128, dtype=NPBF16),
            }
        )
    return in_maps


def run(x, w_qkv, scale, w_out, b_out, trace=False):
    """Run on the 8 NeuronCores; returns (full_output, BassKernelResults)."""
    in_maps = make_in_maps(x, w_qkv, scale, w_out, b_out)
    nc = get_compiled()
    res = run_bass_kernel_spmd(nc, in_maps, core_ids=list(range(NCORES)), trace=trace)
    outs = [res.results[c]["out"].reshape(B_CORE, N, D) for c in range(NCORES)]
    full = np.concatenate(outs, axis=0).astype(np.float32)
    return full, res


def kernel(x, w_qkv, scale, w_out, b_out):
    full, _ = run(x, w_qkv, scale, w_out, b_out, trace=False)
    return full


# revision 51
# speedup vs baseline: 1.0165x; 1.0165x over previous
"""Trainium2 Bass kernel: ViT-style LSA attention (per-head learnable scale,
diagonal self-token mask), data-parallel over batch across 8 NeuronCores.

Reference computation (per batch b of 64, N=197 tokens, D=384, H=8, DH=64):
    qkv = x @ w_qkv ; split q,k,v ; per-head scale on q@k^T scores ;
    diagonal masked to -9.9e8 ; softmax ; attn @ v ; concat heads @ w_out + b.

Sharding: batch 64 -> 8 cores x 8 batches. Weights replicated. No
collectives; host concatenates the per-core outputs.

Device dataflow per core, all TensorE matmuls bf16 (correctness gate is 2e-2
relative error; measured end-to-end bf16 error is ~0.85e-2), fp32 PSUM
accumulation. The whole kernel streams per batch pair / batch so every engine
pipelines; attention heads are processed in quads sharing PSUM banks:

  xT  [384,2N]  PE transposes per batch pair (bf16 PSUM, one evac per j-tile)
  qT,kT [512,2N] = Wq^T/Wk^T @ xT   (Wq pre-scaled on host by the LSA scale)
  v   [N,512]   natural layout per batch; xT slices as stationary operand,
                a constant-1 column appended per head for the softmax sums
  S^T [j,i]     per (b, head-quad): kT stationary, qT moving -> shared PSUM
  P^T = exp(S^T) * mask01  (diag zeroed; max-subtract skipped, |S| < ~20)
  attn-out^T [65,i] = [v|1]-stationary @ P^T  (row 64 = softmax denominator)
  attnT = out^T * reciprocal(denominator)  (broadcast via gpsimd from
                partition 0 — the ucode ignores AP partition offsets on HW)
  out [N,384] = attnT-stationary @ Wo + b

build_nc(reps=R) emits the body R times (per-rep PSUM pool scopes) so HW time
can be measured by wall-clock amplification — this container has no NTFF hook.
"""

import sys

sys.path.insert(0, "/opt/trn_rl_repo")

from contextlib import ExitStack

import ml_dtypes
import numpy as np

import concourse.bass as bass
import concourse.tile as tile
from concourse import bacc, mybir
from concourse.bass_utils import run_bass_kernel_spmd

BF16 = mybir.dt.bfloat16
F32 = mybir.dt.float32
NPBF16 = ml_dtypes.bfloat16

NCORES = 8
B_CORE = 8            # batches per core
N = 197               # tokens per batch
D = 384               # model dim
H = 8                 # heads
DH = 64               # head dim
INNER = H * DH        # 512
T = B_CORE * N        # 1576 tokens per core

# token-chunk starts/sizes for the moving operand of the q/k projections
CHUNKS = [(0, 512), (512, 512), (1024, 512), (1536, T - 1536)]
# per-batch key tiles: (offset, rows)
JTILES = [(0, 128), (128, N - 128)]
# j-tiles over a batch PAIR (transpose/projection granularity)
JTILES2 = [(0, 128), (128, 69), (197, 128), (325, 69)]
# output token tiles
TTILES = [(t0, min(128, T - t0)) for t0 in range(0, T, 128)]

EXP = mybir.ActivationFunctionType.Exp


def build_nc():
    nc = bacc.Bacc("TRN2", target_bir_lowering=False, debug=False)

    x = nc.dram_tensor("x", [T, D], BF16, kind="ExternalInput").ap()
    wq = nc.dram_tensor("wq", [D, INNER], BF16, kind="ExternalInput").ap()
    wk = nc.dram_tensor("wk", [D, INNER], BF16, kind="ExternalInput").ap()
    wv = nc.dram_tensor("wv", [D, INNER], BF16, kind="ExternalInput").ap()
    wo = nc.dram_tensor("wo", [INNER, D], BF16, kind="ExternalInput").ap()
    mask = nc.dram_tensor("mask01", [2, 128, 4, N], BF16, kind="ExternalInput").ap()
    iddr = nc.dram_tensor("ident", [128, 128], BF16, kind="ExternalInput").ap()
    bias = nc.dram_tensor("bias", [128, D], F32, kind="ExternalInput").ap()
    out = nc.dram_tensor("out", [T, D], F32, kind="ExternalOutput").ap()

    with tile.TileContext(nc) as tc, ExitStack() as ctx:
        const = ctx.enter_context(tc.tile_pool(name="const", bufs=1))

        xTs = [
            const.tile([128, T], BF16, tag=f"xT{i}", name=f"xT{i}") for i in range(3)
        ]
        ident = const.tile([128, 128], BF16)
        qT = const.tile([128, 4, T], BF16)
        kT = const.tile([128, 4, T], BF16)
        wq_sb = const.tile([128, 3, INNER], BF16)
        wk_sb = const.tile([128, 3, INNER], BF16)
        wv_sb = const.tile([128, 3, INNER], BF16)
        wo_sb = const.tile([128, 4, D], BF16)
        mk_sb = const.tile([128, 2, 4, N], BF16)
        bi_sb = const.tile([128, D], F32)

        # x tiles ride the SP queue (they gate the transposes feeding every
        # matmul); weights/mask/bias ride the Activation HWDGE queue in parallel
        # identity ships from the host: the gpsimd-built one (memset +
        # affine_select) gates the first PE transpose by ~2us at startup
        nc.scalar.dma_start(out=ident[:], in_=iddr[:])

        vv_pool = ctx.enter_context(tc.tile_pool(name="vv", bufs=3))
        pt_pool = ctx.enter_context(tc.tile_pool(name="pt", bufs=6))
        rp_pool = ctx.enter_context(tc.tile_pool(name="rp", bufs=6))
        rb_pool = ctx.enter_context(tc.tile_pool(name="rb", bufs=6))
        ob_pool = ctx.enter_context(tc.tile_pool(name="ob", bufs=3))

        # ---- x^T via PE transposes, then q/k projections: qT = (x @ Wq)^T ----
        # (own psum scope: its 4 banks are returned before the attention pools open)
        with (
            tc.tile_pool(name="qk_psum", bufs=4, space="PSUM") as qk_psum,
            tc.tile_pool(name="xsb", bufs=3) as x_pool,
        ):
            evac_flip = 0
            for tt, (t0, tsz) in enumerate(TTILES):
                xs = x_pool.tile([128, D], BF16, tag="xs")
                nc.sync.dma_start(out=xs[:tsz, :], in_=x[t0 : t0 + tsz, :])
                if tt == 3:
                    # q/k weights gate the first projections; the rest arrives
                    # before its consumers. One queue, ordered by need.
                    nc.sync.dma_start(
                        out=wq_sb[:], in_=wq.rearrange("(t p) n -> p t n", p=128)
                    )
                    nc.sync.dma_start(
                        out=wk_sb[:], in_=wk.rearrange("(t p) n -> p t n", p=128)
                    )
                if tt == 7:
                    nc.sync.dma_start(
                        out=wv_sb[:], in_=wv.rearrange("(t p) n -> p t n", p=128)
                    )
                if tt == 9:
                    nc.sync.dma_start(
                        out=wo_sb[:], in_=wo.rearrange("(t p) n -> p t n", p=128)
                    )
                if tt == 10:
                    nc.sync.dma_start(
                        out=mk_sb[:], in_=mask.rearrange("t p h n -> p t h n")
                    )
                    nc.sync.dma_start(out=bi_sb[:], in_=bias)
                for kt in range(3):
                    pxp = qk_psum.tile([128, 128], BF16, tag="qk")
                    nc.tensor.transpose(
                        pxp[:, :tsz],
                        xs[:tsz, kt * 128 : (kt + 1) * 128],
                        ident[:tsz, :tsz],
                    )
                    if evac_flip % 3 != 0:
                        nc.scalar.copy(xTs[kt][:, t0 : t0 + tsz], pxp[:, :tsz])
                    else:
                        nc.vector.tensor_copy(xTs[kt][:, t0 : t0 + tsz], pxp[:, :tsz])
                    evac_flip += 1
            for w_sb, dstT in ((wq_sb, qT), (wk_sb, kT)):
                for ft in range(4):
                    for c0, csz in CHUNKS:
                        ps = qk_psum.tile([128, 512], F32, tag="qk")
                        for kt in range(3):
                            nc.tensor.matmul(
                                ps[:, :csz],
                                lhsT=w_sb[:, kt, ft * 128 : (ft + 1) * 128],
                                rhs=xTs[kt][:, c0 : c0 + csz],
                                start=(kt == 0),
                                stop=(kt == 2),
                            )
                        if evac_flip % 3 != 0:
                            nc.scalar.copy(dstT[:, ft, c0 : c0 + csz], ps[:, :csz])
                        else:
                            nc.vector.tensor_copy(
                                dstT[:, ft, c0 : c0 + csz], ps[:, :csz]
                            )
                        evac_flip += 1

        v_psum = ctx.enter_context(tc.tile_pool(name="v_psum", bufs=1, space="PSUM"))
        o_psum = ctx.enter_context(tc.tile_pool(name="o_psum", bufs=1, space="PSUM"))
        d_psum = ctx.enter_context(tc.tile_pool(name="d_psum", bufs=2, space="PSUM"))
        a_psum = ctx.enter_context(tc.tile_pool(name="a_psum", bufs=2, space="PSUM"))
        at_pool = ctx.enter_context(tc.tile_pool(name="at", bufs=3))

        # ---- per-batch: v (natural layout, with ones column), then attention ----
        for b in range(B_CORE):
            t_b = b * N
            aT = at_pool.tile([128, 4, N], BF16, tag="at")
            vv = vv_pool.tile([128, 2, H * 65], BF16, tag="vv")
            for jt, (j0, jsz) in enumerate(JTILES):
                tok0 = t_b + j0
                pv = v_psum.tile([128, 512], F32, tag="v")
                for kt in range(3):
                    nc.tensor.matmul(
                        pv[:jsz, :],
                        lhsT=xTs[kt][:, tok0 : tok0 + jsz],
                        rhs=wv_sb[:, kt, :],
                        start=(kt == 0),
                        stop=(kt == 2),
                    )
                vj = vv[:jsz, jt].rearrange("p (h c) -> p h c", c=65)
                nc.gpsimd.memset(vj[:, :, 64:65], 1.0)
                nc.scalar.copy(
                    vj[:, :, 0:64], pv[:jsz, :].rearrange("p (h c) -> p h c", c=64)
                )

            # head quads sharing one partition half (po): heads (po-parity) with
            # feature tiles 0..3, so exp/mask/recip/broadcast/normalize each run
            # once per quad. PSUM quad tiles use a 256-element head stride so no
            # matmul output crosses a 2KB bank boundary; start/stop flags open
            # and close an accumulation group per bank.
            for quad in ((0, 2, 4, 6), (1, 3, 5, 7)):
                po = (quad[0] % 2) * 64
                pt = pt_pool.tile([128, 2, 4, N], BF16, tag="pt")
                for jt, (j0, jsz) in enumerate(JTILES):
                    tok0 = t_b + j0
                    pd = d_psum.tile([128, 4, 256], F32, tag="d")
                    for hh, h in enumerate(quad):
                        nc.tensor.matmul(
                            pd[:jsz, hh, :N],
                            lhsT=kT[po : po + 64, h // 2, tok0 : tok0 + jsz],
                            rhs=qT[po : po + 64, h // 2, t_b : t_b + N],
                            start=(hh % 2 == 0),
                            stop=(hh % 2 == 1),
                        )
                    nc.scalar.activation(pt[:jsz, jt], pd[:jsz, :, :N], EXP)
                    nc.gpsimd.tensor_mul(pt[:jsz, jt], pt[:jsz, jt], mk_sb[:jsz, jt])
                for pi in range(2):
                    fa = quad[2 * pi] // 2
                    pa = a_psum.tile([65, 2, N], F32, tag="a")
                    for jt, (j0, jsz) in enumerate(JTILES):
                        for hh in (2 * pi, 2 * pi + 1):
                            h = quad[hh]
                            nc.tensor.matmul(
                                pa[:, hh - 2 * pi, :],
                                lhsT=vv[:jsz, jt, h * 65 : (h + 1) * 65],
                                rhs=pt[:jsz, jt, hh, :],
                                start=(jt == 0 and hh % 2 == 0),
                                stop=(jt == 1 and hh % 2 == 1),
                            )
                    # reciprocal lands on partition 0: the partition_broadcast
                    # ucode ignores the AP partition offset (HW-verified)
                    rp = rp_pool.tile([1, 2, N], F32, tag="rp")
                    nc.vector.reciprocal(rp[0:1], pa[64:65])
                    rb = rb_pool.tile([64, 2, N], F32, tag="rb")
                    nc.gpsimd.partition_broadcast(rb[:], rp[0:1])
                    nc.vector.tensor_mul(
                        aT[po : po + 64, fa : fa + 2, :], pa[0:64], rb[:]
                    )

            # ---- output projection + bias for this batch (overlaps the next
            # batch's attention; aT tile is released back to the pool) ----
            for j0, jsz in JTILES:
                pp = o_psum.tile([128, D], F32, tag="o")
                for kf in range(4):
                    nc.tensor.matmul(
                        pp[:jsz, :],
                        lhsT=aT[:, kf, j0 : j0 + jsz],
                        rhs=wo_sb[:, kf, :],
                        start=(kf == 0),
                        stop=(kf == 3),
                    )
                ob = ob_pool.tile([128, D], F32, tag="ob")
                nc.vector.tensor_add(ob[:jsz, :], pp[:jsz, :], bi_sb[:jsz, :])
                nc.sync.dma_start(
                    out=out[t_b + j0 : t_b + j0 + jsz, :], in_=ob[:jsz, :]
                )

    return nc


_CACHE: dict = {}


def get_compiled():
    if "nc" not in _CACHE:
        nc = build_nc()
        nc.compile()
        _CACHE["nc"] = nc
    return _CACHE["nc"]


def make_in_maps(x, w_qkv, scale, w_out, b_out):
    x = np.asarray(x, np.float32)
    w_qkv = np.asarray(w_qkv, np.float32)
    scale = np.asarray(scale, np.float32)
    w_out = np.asarray(w_out, np.float32)
    b_out = np.asarray(b_out, np.float32)

    # fold the per-head LSA scale into Wq (exact in real arithmetic; the
    # scores become (x @ (Wq*s)) @ k^T = s * (q @ k^T))
    scale_rep = np.repeat(scale, DH)  # [512]
    wq = (w_qkv[:, :INNER] * scale_rep[None, :]).astype(NPBF16)
    wk = w_qkv[:, INNER : 2 * INNER].astype(NPBF16)
    wv = w_qkv[:, 2 * INNER :].astype(NPBF16)
    wo = w_out.astype(NPBF16)
    bias = np.ascontiguousarray(np.broadcast_to(b_out, (128, D)))

    mask = np.ones((2, 128, N), np.float32)
    for t in range(2):
        for j in range(128):
            g = t * 128 + j
            if g < N:
                mask[t, j, g] = 0.0
    # duplicated along a head-quad axis: one gpsimd multiply masks four heads
    mask = np.repeat(mask[:, :, None, :], 4, axis=2).astype(NPBF16)

    xs = x.reshape(NCORES, B_CORE, N, D)
    in_maps = []
    for c in range(NCORES):
        in_maps.append(
            {
                "x": np.ascontiguousarray(xs[c].reshape(T, D)).astype(NPBF16),
                "wq": wq,
                "wk": wk,
                "wv": wv,
                "wo": wo,
                "mask01": mask,
                "bias": bias,
                "ident": np.eye(128, dtype=NPBF16),
            }
        )
    return in_maps


def run(x, w_qkv, scale, w_out, b_out, trace=False):
    """Run on the 8 NeuronCores; returns (full_output, BassKernelResults)."""
    in_maps = make_in_maps(x, w_qkv, scale, w_out, b_out)
    nc = get_compiled()
    res = run_bass_kernel_spmd(nc, in_maps, core_ids=list(range(NCORES)), trace=trace)
    outs = [res.results[c]["out"].reshape(B_CORE, N, D) for c in range(NCORES)]
    full = np.concatenate(outs, axis=0).astype(np.float32)
    return full, res


def kernel(x, w_qkv, scale, w_out, b_out):
    full, _ = run(x, w_qkv, scale, w_out, b_out, trace=False)
    return full
